# revision 1
# baseline (speedup 1.0000x reference)
"""DeltaNet block kernel for 8 Trainium2 NeuronCores.

Sharding: core c -> (batch b = c//2, head-group hg = c%2, 6 heads each).
Kernel 1: rmsnorm -> q/k/v/g/beta/a projections -> short conv -> l2norm ->
          chunked gated delta rule (L=128, 16-term Neumann triangular solve)
          -> gated head RMSNorm -> partial o-projection  => po[b,hg]
Host:     h = x + po[b,0] + po[b,1]
Kernel 2: token-sharded FFN: out = h + (silu(hn@w1)*(hn@w3))@w2
"""
import os
from contextlib import ExitStack

import numpy as np

os.environ["BASS_NEVER_TRACE"] = "1"  # no NTFF hook under this axon client
import ml_dtypes

import concourse.bass as bass
import concourse.mybir as mybir
import concourse.tile as tile
from concourse import bacc
from concourse.bass_utils import run_bass_kernel_spmd
from concourse.masks import make_identity, make_upper_triangular

F32 = mybir.dt.float32
F32R = mybir.dt.float32r
BF16 = mybir.dt.bfloat16
AF = mybir.ActivationFunctionType
ALU = mybir.AluOpType

B, T, DIM = 4, 4096, 1024
H, DK, DV = 12, 64, 128
HL = 6              # local heads per core
L = 128             # delta chunk length
SEG = 256           # tokens per segment
FFN = 2816
EPS = 1e-5
NCAT = 2342         # q(384) k(384) v(768) g(768) beta(6)@2304 a(6)@2336

bf = lambda a: np.ascontiguousarray(a).astype(ml_dtypes.bfloat16)
f32 = lambda a: np.ascontiguousarray(a, dtype=np.float32)


def r32(ap):
    return ap.bitcast(F32R)


# ----------------------------------------------------------------------------
# Kernel 1 builder
# ----------------------------------------------------------------------------
SKIP_DELTA = False
SKIP_OPROJ = False


def build_k1(Ttok):
    nseg = Ttok // SEG
    ncps = SEG // L  # chunks per segment
    nc = bacc.Bacc("TRN2", target_bir_lowering=False, debug=False, num_devices=8)

    x_d = nc.dram_tensor("x", [Ttok, DIM], F32, kind="ExternalInput")
    wcat_d = nc.dram_tensor("wcat", [DIM, NCAT], BF16, kind="ExternalInput")
    wbahi_d = nc.dram_tensor("wbahi", [DIM, 38], BF16, kind="ExternalInput")
    walo_d = nc.dram_tensor("walo", [DIM, 38], BF16, kind="ExternalInput")
    convw_d = nc.dram_tensor("convw", [1536, 4], F32, kind="ExternalInput")
    dtb_d = nc.dram_tensor("dtb", [38, 1], F32, kind="ExternalInput")
    negA_d = nc.dram_tensor("negA", [38, 1], F32, kind="ExternalInput")
    onw_d = nc.dram_tensor("onw", [128, 1], F32, kind="ExternalInput")
    wo_d = nc.dram_tensor("wo", [768, DIM], BF16, kind="ExternalInput")
    po_d = nc.dram_tensor("po", [Ttok, DIM], F32, kind="ExternalOutput")

    with tile.TileContext(nc) as tc, ExitStack() as ctx:
        cons = ctx.enter_context(tc.tile_pool(name="cons", bufs=1))
        wgt = ctx.enter_context(tc.tile_pool(name="wgt", bufs=1))
        xp = ctx.enter_context(tc.tile_pool(name="xp", bufs=2))
        segp = ctx.enter_context(tc.tile_pool(name="segp", bufs=2))
        segq = ctx.enter_context(tc.tile_pool(name="segq", bufs=1))
        ch = ctx.enter_context(tc.tile_pool(name="ch", bufs=3))
        sp = ctx.enter_context(tc.tile_pool(name="sp", bufs=1))
        psA = ctx.enter_context(tc.tile_pool(name="psA", bufs=1, space="PSUM"))
        ps19p = ctx.enter_context(tc.tile_pool(name="ps19", bufs=1, space="PSUM"))
        psB = ctx.enter_context(tc.tile_pool(name="psB", bufs=1, space="PSUM"))
        _pctr = [0]

        def pstile(dtype=F32):
            t = psB.tile([128, 256], dtype, tag=f"ps{_pctr[0] % 6}",
                         name=f"psr{_pctr[0]}")
            _pctr[0] += 1
            return t
        drp = ctx.enter_context(tc.tile_pool(name="drp", bufs=2, space="DRAM"))

        # ---- constants ----
        id128f = cons.tile([128, 128], F32)
        make_identity(nc, id128f[:])
        id128b = cons.tile([128, 128], BF16)
        make_identity(nc, id128b[:])
        mku_s = cons.tile([128, 128], F32)   # strict upper ones
        make_upper_triangular(nc, mku_s[:], val=1.0, diag=False)
        mku_i = cons.tile([128, 128], F32)   # inclusive upper ones
        make_upper_triangular(nc, mku_i[:], val=1.0, diag=True)
        blk2 = cons.tile([128, 2], F32)
        nc.vector.memset(blk2[:], 0.0)
        nc.vector.memset(blk2[0:64, 0:1], 1.0)
        nc.vector.memset(blk2[64:128, 1:2], 1.0)
        zero12 = cons.tile([38, 128], F32)
        nc.vector.memset(zero12[:], 0.0)
        epsc = cons.tile([128, 1], F32)
        nc.vector.memset(epsc[:], EPS)
        epsq = cons.tile([128, 1], F32)
        nc.vector.memset(epsq[:], float(DK) * 1e-6)
        epsk = cons.tile([128, 1], F32)
        nc.vector.memset(epsk[:], 1e-6)

        # ---- weights to SBUF ----
        wcat = wgt.tile([128, 8, NCAT], BF16)
        nc.sync.dma_start(out=wcat[:], in_=wcat_d[:].rearrange("(a p) c -> p a c", p=128))
        wbahi = wgt.tile([128, 8, 38], BF16)
        nc.sync.dma_start(out=wbahi[:], in_=wbahi_d[:].rearrange("(a p) c -> p a c", p=128))
        walo = wgt.tile([128, 8, 38], BF16)
        nc.sync.dma_start(out=walo[:], in_=walo_d[:].rearrange("(a p) c -> p a c", p=128))
        convw = wgt.tile([128, 12, 4], F32)
        nc.sync.dma_start(out=convw[:], in_=convw_d[:].rearrange("(a p) c -> p a c", p=128))
        dtb = wgt.tile([38, 1], F32)
        nc.sync.dma_start(out=dtb[:], in_=dtb_d[:])
        negA = wgt.tile([38, 1], F32)
        nc.sync.dma_start(out=negA[:], in_=negA_d[:])
        onw = wgt.tile([128, 1], F32)
        nc.sync.dma_start(out=onw[:], in_=onw_d[:])
        wo = wgt.tile([128, 6, DIM], BF16)
        nc.sync.dma_start(out=wo[:], in_=wo_d[:].rearrange("(a p) c -> p a c", p=128))

        # persistent delta states (ping-pong per head)
        S = [[sp.tile([64, DV], BF16, tag=f"S{h}_{pp}", name=f"S{h}_{pp}")
              for pp in range(2)] for h in range(HL)]
        for h in range(HL):
            nc.vector.memset(S[h][0][:], 0.0)

        # conv halo carry
        halo = sp.tile([128, 12, 3], BF16, tag="halo")
        nc.vector.memset(halo[:], 0.0)

        for s in range(nseg):
            # ============ x load + rmsnorm + transpose ============
            xnTh = segp.tile([128, 8, SEG], BF16, tag="xnTh")
            xnTl = segq.tile([128, 8, SEG], BF16, tag="xnTl")
            for t4 in range(SEG // 128):
                tt = s * (SEG // 128) + t4
                xt = xp.tile([128, DIM], F32, tag="xt")
                nc.sync.dma_start(out=xt[:], in_=x_d[tt * 128:(tt + 1) * 128, :])
                xsq = xp.tile([128, DIM], F32, tag="xsq")
                ssq = xp.tile([128, 1], F32, tag="ssq")
                nc.scalar.activation(out=xsq[:], in_=xt[:], func=AF.Square,
                                     accum_out=ssq[:])
                rst = xp.tile([128, 1], F32, tag="rst")
                nc.scalar.activation(out=rst[:], in_=ssq[:], func=AF.Ln,
                                     scale=1.0 / DIM, bias=epsc[:])
                nc.scalar.activation(out=rst[:], in_=rst[:], func=AF.Exp,
                                     scale=-0.5)
                xn = xp.tile([128, DIM], F32, tag="xn")
                nc.scalar.activation(out=xn[:], in_=xt[:], func=AF.Copy, scale=rst[:])
                for kc in range(8):
                    pt = pstile(F32)
                    nc.tensor.transpose(pt[:, 0:128], xn[:, kc * 128:(kc + 1) * 128],
                                        id128f[:])
                    cs = slice(t4 * 128, t4 * 128 + 128)
                    nc.scalar.activation(out=xnTh[:, kc, cs], in_=pt[:, 0:128],
                                         func=AF.Copy)
                    nc.vector.tensor_sub(xnTl[:, kc, cs], pt[:, 0:128],
                                         xnTh[:, kc, cs])

            # ============ projections ============
            qkvb = segq.tile([128, 12, SEG + 3], BF16, tag="qkvb")
            nc.scalar.activation(out=qkvb[:, :, 0:3], in_=halo[:], func=AF.Copy)
            gateT = segq.tile([128, 6, SEG], BF16, tag="gateT")
            for jcol in range(18):
                c0 = jcol * 128
                pj = psA.tile([128, SEG], F32, tag="psA")
                for kc in range(8):
                    nc.tensor.matmul(pj[:], wcat[:, kc, c0:c0 + 128],
                                     xnTh[:, kc, :], start=(kc == 0), stop=(kc == 7))
                if jcol < 12:
                    nc.scalar.activation(out=qkvb[:, jcol, 3:SEG + 3], in_=pj[:],
                                         func=AF.Copy)
                else:
                    nc.scalar.activation(out=gateT[:, jcol - 12, :], in_=pj[:],
                                         func=AF.Silu)
            # beta/a columns with low-precision corrections
            p19 = ps19p.tile([38, SEG], F32, tag="p19")
            for kc in range(8):
                nc.tensor.matmul(p19[:], wcat[:, kc, 2304:2342], xnTh[:, kc, :],
                                 start=(kc == 0), stop=False)
            for kc in range(8):
                nc.tensor.matmul(p19[:], wbahi[:, kc, :], xnTl[:, kc, :],
                                 start=False, stop=False)
            for kc in range(8):
                nc.tensor.matmul(p19[:], walo[:, kc, :], xnTh[:, kc, :],
                                 start=False, stop=(kc == 7))
            ba = segq.tile([38, SEG], F32, tag="ba")
            nc.scalar.activation(out=ba[:], in_=p19[:], func=AF.Copy)

            # ============ conv + silu ============
            csil = segp.tile([128, 12, SEG], BF16, tag="csil")
            cacc = segq.tile([128, 12, SEG], BF16, tag="cacc")
            ctmp = segq.tile([128, 12, SEG], BF16, tag="ctmp")
            nc.vector.tensor_mul(cacc[:], qkvb[:, :, 3:SEG + 3],
                                 convw[:, :, 3:4].to_broadcast((128, 12, SEG)))
            for i in (2, 1, 0):
                nc.vector.tensor_mul(ctmp[:], qkvb[:, :, i:i + SEG],
                                     convw[:, :, i:i + 1].to_broadcast((128, 12, SEG)))
                nc.vector.tensor_add(cacc[:], cacc[:], ctmp[:])
            nc.scalar.activation(out=halo[:], in_=qkvb[:, :, SEG:SEG + 3], func=AF.Copy)
            nc.scalar.activation(out=csil[:], in_=cacc[:], func=AF.Silu)

            # ============ l2norm scales for q/k ============
            sqt = segq.tile([128, SEG], F32, tag="sqt")
            rp = []
            for t in range(6):
                nc.scalar.activation(out=sqt[:], in_=csil[:, t, :], func=AF.Square)
                pq = pstile(F32)
                nc.tensor.matmul(pq[0:2, 0:SEG], blk2[:], sqt[:],
                                 start=True, stop=True)
                rpt = segp.tile([2, SEG], F32, tag=f"rp{t}", name=f"rp{t}")
                if t < 3:
                    nc.scalar.activation(out=rpt[:], in_=pq[0:2, 0:SEG], func=AF.Ln,
                                         scale=float(DK), bias=epsq[0:2, :])
                else:
                    nc.scalar.activation(out=rpt[:], in_=pq[0:2, 0:SEG], func=AF.Ln,
                                         scale=1.0, bias=epsk[0:2, :])
                nc.scalar.activation(out=rpt[:], in_=rpt[:], func=AF.Exp,
                                     scale=-0.5)
                rp.append(rpt)

            # plain-scaled q/k (channel-major)
            Qts = segp.tile([128, 3, SEG], BF16, tag="Qts")
            Kts = segp.tile([128, 3, SEG], BF16, tag="Kts")
            bcq = segq.tile([128, SEG], F32, tag="bcq")
            bck = segq.tile([128, SEG], F32, tag="bck")
            for t in range(3):
                rqd = drp.tile([2, SEG], F32, tag="rqd")
                nc.sync.dma_start(out=rqd[:], in_=rp[t][:])
                rkd = drp.tile([2, SEG], F32, tag="rkd")
                nc.sync.dma_start(out=rkd[:], in_=rp[3 + t][:])
                for i in range(2):
                    hh = slice(64 * i, 64 * i + 64)
                    nc.sync.dma_start(out=bcq[hh, :], in_=rqd[i:i + 1, :].to_broadcast((64, SEG)))
                    nc.sync.dma_start(out=bck[hh, :], in_=rkd[i:i + 1, :].to_broadcast((64, SEG)))
                nc.vector.tensor_mul(Qts[:, t, :], csil[:, t, :], bcq[:])
                nc.vector.tensor_mul(Kts[:, t, :], csil[:, 3 + t, :], bck[:])

            # ============ delta chunks ============
            gato = segp.tile([128, 6, SEG], BF16, tag="gato")
            for cc in ([] if SKIP_DELTA else range(ncps)):
                csl = slice(cc * L, (cc + 1) * L)
                cglob = s * ncps + cc

                # ---- beta / g / gc pipeline for this chunk ----
                spg = ch.tile([38, 128], F32, tag="spg")
                gcsg = ch.tile([38, 128], F32, tag="gcsg")
                nc.scalar.activation(out=gcsg[0:6, :], in_=ba[0:6, csl],
                                     func=AF.Exp, scale=-1.0)
                nc.vector.tensor_scalar(out=gcsg[0:6, :], in0=gcsg[0:6, :],
                                        scalar1=1.0, scalar2=None, op0=ALU.add)
                nc.vector.reciprocal(out=gcsg[0:6, :], in_=gcsg[0:6, :])
                nc.scalar.activation(out=spg[32:38, :], in_=ba[32:38, csl],
                                     func=AF.Exp, bias=dtb[32:38, :])
                nc.scalar.activation(out=spg[32:38, :], in_=spg[32:38, :],
                                     func=AF.Ln, bias=1.0)
                grow = ch.tile([38, 128], F32, tag="grow")
                nc.vector.tensor_scalar(out=grow[32:38, :], in0=spg[32:38, :],
                                        scalar1=negA[32:38, :], scalar2=None,
                                        op0=ALU.mult)
                nc.vector.tensor_tensor_scan(out=gcsg[32:38, :], data0=grow[32:38, :],
                                             data1=zero12[32:38, :], initial=0.0,
                                             op0=ALU.add, op1=ALU.add)
                ptb = pstile(F32)
                nc.tensor.transpose(ptb[:, 0:38], gcsg[:], id128f[0:38, 0:38])
                bgt = ch.tile([128, 38], F32, tag="bgt")
                nc.scalar.activation(out=bgt[:], in_=ptb[:, 0:38], func=AF.Copy)
                # gc rows to DRAM once; replicate rows and last-token column back
                gcd = drp.tile([6, 128], F32, tag="gcd")
                nc.sync.dma_start(out=gcd[:], in_=gcsg[32:38, :])
                gcrep6 = ch.tile([128, 6, 128], F32, tag="gcrep6")
                nc.sync.dma_start(
                    out=gcrep6[:],
                    in_=bass.AP(tensor=gcd.tensor, offset=gcd.offset,
                                ap=[[0, 128], [128, 6], [1, 128]]))
                gamc = ch.tile([128, 6], F32, tag="gamc")
                nc.scalar.activation(out=gamc[:], in_=bgt[:, 32:38], func=AF.Exp)
                gclr = ch.tile([128, 6], F32, tag="gclr")
                nc.sync.dma_start(
                    out=gclr[:],
                    in_=bass.AP(tensor=gcd.tensor, offset=gcd.offset + 127,
                                ap=[[0, 128], [128, 6]]))
                dtmp = ch.tile([128, 6], F32, tag="dtmp")
                nc.vector.tensor_sub(dtmp[:], gclr[:], bgt[:, 32:38])
                dcola = ch.tile([128, 6], F32, tag="dcola")
                nc.scalar.activation(out=dcola[:], in_=dtmp[:], func=AF.Exp)
                gamls = ch.tile([128, 6], F32, tag="gamls")
                nc.scalar.activation(out=gamls[:], in_=gclr[:], func=AF.Exp)

                # q/k token-major pairs
                ktokp = ch.tile([128, 3, 128], BF16, tag="ktokp")
                qtokp = ch.tile([128, 3, 128], BF16, tag="qtokp")
                for t in range(3):
                    pkt = pstile(BF16)
                    nc.tensor.transpose(pkt[:, 0:128], Kts[:, t, csl], id128b[:])
                    nc.scalar.activation(out=ktokp[:, t, :], in_=pkt[:, 0:128],
                                         func=AF.Copy)
                    pqt = pstile(BF16)
                    nc.tensor.transpose(pqt[:, 0:128], Qts[:, t, csl], id128b[:])
                    nc.scalar.activation(out=qtokp[:, t, :], in_=pqt[:, 0:128],
                                         func=AF.Copy)
                # Gamma-scaled q, back to channel-major at partition base 0
                qgch = []
                for h2 in range(HL):
                    t2, half2 = h2 // 2, h2 % 2
                    qtg = ch.tile([128, 64], BF16, tag="qtg", name="qtg")
                    nc.vector.tensor_scalar(out=qtg[:],
                                            in0=qtokp[:, t2, 64 * half2:64 * half2 + 64],
                                            scalar1=gamc[:, h2:h2 + 1], scalar2=None,
                                            op0=ALU.mult)
                    pqg = pstile(BF16)
                    nc.tensor.transpose(pqg[0:64, 0:128], qtg[:], id128b[:])
                    qg = ch.tile([64, 128], BF16, tag=f"qg{h2}", name=f"qg{h2}")
                    nc.scalar.activation(out=qg[:], in_=pqg[0:64, 0:128], func=AF.Copy)
                    qgch.append(qg)

                for h in range(HL):
                    t, half = h // 2, h % 2
                    hh = slice(64 * half, 64 * half + 64)
                    Ksl = Kts[hh, t, csl]
                    Qsl = Qts[hh, t, csl]
                    Qgsl = qgch[h][:]
                    Ktok = ktokp[:, t, 64 * half:64 * half + 64]
                    Sprev = S[h][cglob % 2]
                    Snext = S[h][(cglob + 1) % 2]

                    # masked KK^T and KQ^T
                    pkk = pstile(F32)
                    nc.tensor.matmul(pkk[:, 0:128], Ksl, Ksl, start=True, stop=True)
                    Msb = ch.tile([128, 128], F32, tag="Msb")
                    nc.vector.tensor_mul(Msb[:], mku_s[:], pkk[:, 0:128])
                    pkq = pstile(F32)
                    nc.tensor.matmul(pkq[:, 0:128], Ksl, Qsl, start=True, stop=True)
                    KQm = ch.tile([128, 128], F32, tag="KQm")
                    nc.vector.tensor_mul(KQm[:], mku_i[:], pkq[:, 0:128])

                    # decay matrix Db[i,t] = exp(min(gc_t - gc_i, 0))
                    Db = ch.tile([128, 128], F32, tag="Db")
                    nc.vector.tensor_scalar(out=Db[:], in0=gcrep6[:, h, :],
                                            scalar1=bgt[:, 32 + h:33 + h],
                                            scalar2=0.0, op0=ALU.subtract,
                                            op1=ALU.min)
                    nc.scalar.activation(out=Db[:], in_=Db[:], func=AF.Exp)

                    # Abar = beta_i * Db * M ; Gbar = Db * KQ
                    Ab = ch.tile([128, 128], BF16, tag="Ab")
                    nc.vector.scalar_tensor_tensor(out=Ab[:], in0=Db[:],
                                                   scalar=bgt[:, h:h + 1], in1=Msb[:],
                                                   op0=ALU.mult, op1=ALU.mult)
                    Gb = ch.tile([128, 128], BF16, tag="Gb")
                    nc.vector.tensor_mul(Gb[:], Db[:], KQm[:])

                    # 16-term Neumann inverse factors
                    pw = pstile(BF16)
                    At = ch.tile([128, 128], BF16, tag="At")
                    nc.tensor.transpose(pw[:, 0:128], Ab[:], id128b[:])
                    nc.scalar.activation(out=At[:], in_=pw[:, 0:128], func=AF.Copy)
                    pw2 = pstile(F32)
                    nc.tensor.matmul(pw2[:, 0:128], At[:], Ab[:], start=True, stop=True)
                    A2p = ch.tile([128, 128], BF16, tag="A2p")
                    A2i = ch.tile([128, 128], BF16, tag="A2i")
                    nc.scalar.activation(out=A2p[:], in_=pw2[:, 0:128], func=AF.Copy)
                    nc.vector.tensor_add(A2i[:], id128b[:], pw2[:, 0:128])
                    pw3 = pstile(F32)
                    nc.tensor.matmul(pw3[:, 0:128], Ab[:], At[:], start=True, stop=True)
                    T2p = ch.tile([128, 128], BF16, tag="T2p")
                    nc.scalar.activation(out=T2p[:], in_=pw3[:, 0:128], func=AF.Copy)
                    pw4 = pstile(F32)
                    nc.tensor.matmul(pw4[:, 0:128], T2p[:], A2p[:], start=True, stop=True)
                    A4p = ch.tile([128, 128], BF16, tag="A4p")
                    A4i = ch.tile([128, 128], BF16, tag="A4i")
                    nc.scalar.activation(out=A4p[:], in_=pw4[:, 0:128], func=AF.Copy)
                    nc.vector.tensor_add(A4i[:], id128b[:], pw4[:, 0:128])
                    pw5 = pstile(F32)
                    nc.tensor.matmul(pw5[:, 0:128], A2p[:], T2p[:], start=True, stop=True)
                    T4p = ch.tile([128, 128], BF16, tag="T4p")
                    nc.scalar.activation(out=T4p[:], in_=pw5[:, 0:128], func=AF.Copy)
                    pw6 = pstile(F32)
                    nc.tensor.matmul(pw6[:, 0:128], T4p[:], A4p[:], start=True, stop=True)
                    A8i = ch.tile([128, 128], BF16, tag="A8i")
                    nc.vector.tensor_add(A8i[:], id128b[:], pw6[:, 0:128])
                    F0 = ch.tile([128, 128], BF16, tag="F0")
                    nc.vector.tensor_sub(F0[:], id128b[:], Ab[:])

                    # X0 = [Vtok | Ktok*Gamma]
                    X0 = ch.tile([128, 192], BF16, tag="X0")
                    pvt = pstile(BF16)
                    nc.tensor.transpose(pvt[:, 0:128], csil[:, 6 + h, csl], id128b[:])
                    nc.scalar.activation(out=X0[:, 0:128], in_=pvt[:, 0:128],
                                         func=AF.Copy)
                    nc.vector.tensor_scalar(out=X0[:, 128:192], in0=Ktok,
                                            scalar1=gamc[:, h:h + 1], scalar2=None,
                                            op0=ALU.mult)

                    # apply chain: X4 = (I-A)(I+A2)(I+A4)(I+A8) X0
                    px1 = pstile(F32)
                    nc.tensor.matmul(px1[:, 0:192], A8i[:], X0[:], start=True, stop=True)
                    X1 = ch.tile([128, 192], BF16, tag="X1")
                    nc.scalar.activation(out=X1[:], in_=px1[:, 0:192], func=AF.Copy)
                    px2 = pstile(F32)
                    nc.tensor.matmul(px2[:, 0:192], A4i[:], X1[:], start=True, stop=True)
                    X2 = ch.tile([128, 192], BF16, tag="X2")
                    nc.vector.tensor_copy(X2[:], px2[:, 0:192])
                    px3 = pstile(F32)
                    nc.tensor.matmul(px3[:, 0:192], A2i[:], X2[:], start=True, stop=True)
                    X3 = ch.tile([128, 192], BF16, tag="X3")
                    nc.scalar.activation(out=X3[:], in_=px3[:, 0:192], func=AF.Copy)
                    px4 = pstile(F32)
                    nc.tensor.matmul(px4[:, 0:192], F0[:], X3[:], start=True, stop=True)
                    YJb = ch.tile([128, 192], BF16, tag="YJb")
                    nc.scalar.activation(out=YJb[:], in_=px4[:, 0:192], func=AF.Copy,
                                         scale=bgt[:, h:h + 1])

                    # U = Yb - Jb S0
                    pjt = pstile(BF16)
                    nc.tensor.transpose(pjt[0:64, 0:128], YJb[:, 128:192], id128b[:])
                    nJT = ch.tile([64, 128], BF16, tag="nJT")
                    nc.scalar.activation(out=nJT[:], in_=pjt[0:64, 0:128],
                                         func=AF.Copy, scale=-1.0)
                    pU = pstile(F32)
                    nc.tensor.matmul(pU[:, 0:128], nJT[:], Sprev[:], start=True,
                                     stop=True)
                    Usb = ch.tile([128, 128], BF16, tag="Usb")
                    nc.vector.tensor_add(Usb[:], pU[:, 0:128], YJb[:, 0:128])

                    # O = Qg S0 + G U (token-major), normalize, gate
                    pO = pstile(F32)
                    nc.tensor.matmul(pO[:, 0:128], Qgsl, Sprev[:], start=True,
                                     stop=False)
                    nc.tensor.matmul(pO[:, 0:128], Gb[:], Usb[:], start=False,
                                     stop=True)
                    osc = ch.tile([128, 128], F32, tag="osc")
                    ossq = ch.tile([128, 1], F32, tag="ossq")
                    nc.scalar.activation(out=osc[:], in_=pO[:, 0:128], func=AF.Square,
                                         accum_out=ossq[:])
                    orst = ch.tile([128, 1], F32, tag="orst")
                    nc.scalar.activation(out=orst[:], in_=ossq[:], func=AF.Ln,
                                         scale=1.0 / DV, bias=epsc[:])
                    nc.scalar.activation(out=orst[:], in_=orst[:], func=AF.Exp,
                                         scale=-0.5)
                    On = ch.tile([128, 128], BF16, tag="On")
                    nc.scalar.activation(out=On[:], in_=pO[:, 0:128], func=AF.Copy,
                                         scale=orst[:])
                    pot = pstile(BF16)
                    nc.tensor.transpose(pot[:, 0:128], On[:], id128b[:])
                    nc.vector.scalar_tensor_tensor(out=gato[:, h, csl],
                                                   in0=pot[:, 0:128], scalar=onw[:],
                                                   in1=gateT[:, h, csl],
                                                   op0=ALU.mult, op1=ALU.mult)

                    # S update: Snext = GamL*Sprev + Kbar^T U
                    Kb = ch.tile([128, 64], BF16, tag="Kb")
                    nc.vector.tensor_scalar(out=Kb[:], in0=Ktok,
                                            scalar1=dcola[:, h:h + 1], scalar2=None,
                                            op0=ALU.mult)
                    pS = pstile(F32)
                    nc.tensor.matmul(pS[0:64, 0:128], Kb[:], Usb[:], start=True,
                                     stop=True)
                    nc.vector.scalar_tensor_tensor(out=Snext[:], in0=Sprev[:],
                                                   scalar=gamls[0:64, h:h + 1],
                                                   in1=pS[0:64, 0:128],
                                                   op0=ALU.mult, op1=ALU.add)

            # ============ o-projection ============
            for t4 in ([] if SKIP_OPROJ else range(SEG // 128)):
                tsl = slice(t4 * 128, t4 * 128 + 128)
                tt = s * (SEG // 128) + t4
                post = xp.tile([128, DIM], F32, tag="post")
                for n in range(2):
                    pp = psA.tile([128, 512], F32, tag="psA")
                    for j in range(6):
                        nc.tensor.matmul(pp[:], gato[:, j, tsl],
                                         wo[:, j, n * 512:(n + 1) * 512],
                                         start=(j == 0), stop=(j == 5))
                    nc.scalar.activation(out=post[:, n * 512:(n + 1) * 512],
                                         in_=pp[:], func=AF.Copy)
                nc.sync.dma_start(out=po_d[tt * 128:(tt + 1) * 128, :], in_=post[:])

    nc.compile()
    return nc


# ----------------------------------------------------------------------------
# Kernel 2 builder (FFN)
# ----------------------------------------------------------------------------
def build_k2(Ttok):
    nc = bacc.Bacc("TRN2", target_bir_lowering=False, debug=False, num_devices=8)
    h_d = nc.dram_tensor("h", [Ttok, DIM], F32, kind="ExternalInput")
    w13_d = nc.dram_tensor("w13", [DIM, 2 * FFN], BF16, kind="ExternalInput")
    w2_d = nc.dram_tensor("w2", [FFN, DIM], BF16, kind="ExternalInput")
    out_d = nc.dram_tensor("out", [Ttok, DIM], F32, kind="ExternalOutput")
    NB = FFN // 256  # 11 paired column blocks

    with tile.TileContext(nc) as tc, ExitStack() as ctx:
        cons = ctx.enter_context(tc.tile_pool(name="cons", bufs=1))
        wgt = ctx.enter_context(tc.tile_pool(name="wgt", bufs=1))
        tp = ctx.enter_context(tc.tile_pool(name="tp", bufs=2))
        ps1 = ctx.enter_context(tc.tile_pool(name="ps1", bufs=4, space="PSUM"))
        ps2 = ctx.enter_context(tc.tile_pool(name="ps2", bufs=2, space="PSUM"))

        id128b = cons.tile([128, 128], BF16)
        make_identity(nc, id128b[:])
        id128f = cons.tile([128, 128], F32)
        make_identity(nc, id128f[:])
        epsc = cons.tile([128, 1], F32)
        nc.vector.memset(epsc[:], EPS)

        w13 = wgt.tile([128, 8, 2 * FFN], BF16)
        nc.sync.dma_start(out=w13[:], in_=w13_d[:].rearrange("(a p) c -> p a c", p=128))
        w2 = wgt.tile([128, 22, DIM], BF16)
        nc.sync.dma_start(out=w2[:], in_=w2_d[:].rearrange("(a p) c -> p a c", p=128))

        for tt in range(Ttok // 128):
            ht = tp.tile([128, DIM], F32, tag="ht")
            nc.sync.dma_start(out=ht[:], in_=h_d[tt * 128:(tt + 1) * 128, :])
            hsq = tp.tile([128, DIM], F32, tag="hsq")
            ssq = tp.tile([128, 1], F32, tag="ssq")
            nc.scalar.activation(out=hsq[:], in_=ht[:], func=AF.Square,
                                 accum_out=ssq[:])
            rst = tp.tile([128, 1], F32, tag="rst")
            nc.scalar.activation(out=rst[:], in_=ssq[:], func=AF.Ln,
                                 scale=1.0 / DIM, bias=epsc[:])
            nc.scalar.activation(out=rst[:], in_=rst[:], func=AF.Exp,
                                 scale=-0.5)
            hn = tp.tile([128, DIM], F32, tag="hn")
            nc.scalar.activation(out=hn[:], in_=ht[:], func=AF.Copy, scale=rst[:])
            hnT = tp.tile([128, 8, 128], BF16, tag="hnT")
            for kc in range(8):
                pt = ps1.tile([128, 256], F32, tag="ps")
                nc.tensor.transpose(pt[:, 0:128], hn[:, kc * 128:(kc + 1) * 128],
                                    id128f[:])
                nc.scalar.activation(out=hnT[:, kc, :], in_=pt[:, 0:128], func=AF.Copy)

            act = tp.tile([128, FFN], BF16, tag="act")
            for j in range(NB):
                p1 = ps1.tile([128, 256], F32, tag="ps")
                p3 = ps1.tile([128, 256], F32, tag="ps")
                c0 = j * 512
                for kc in range(8):
                    nc.tensor.matmul(p1[:], hnT[:, kc, :], w13[:, kc, c0:c0 + 256],
                                     start=(kc == 0), stop=(kc == 7))
                for kc in range(8):
                    nc.tensor.matmul(p3[:], hnT[:, kc, :],
                                     w13[:, kc, c0 + 256:c0 + 512],
                                     start=(kc == 0), stop=(kc == 7))
                sl1 = tp.tile([128, 256], BF16, tag="sl1")
                nc.scalar.activation(out=sl1[:], in_=p1[:], func=AF.Silu)
                nc.vector.scalar_tensor_tensor(out=act[:, j * 256:(j + 1) * 256],
                                               in0=p3[:], scalar=1.0, in1=sl1[:],
                                               op0=ALU.mult, op1=ALU.mult)
            actT = tp.tile([128, 22, 128], BF16, tag="actT")
            for kc in range(22):
                pt = ps1.tile([128, 256], BF16, tag="ps")
                nc.tensor.transpose(pt[:, 0:128], act[:, kc * 128:(kc + 1) * 128],
                                    id128b[:])
                nc.scalar.activation(out=actT[:, kc, :], in_=pt[:, 0:128],
                                     func=AF.Copy)
            ot = tp.tile([128, DIM], F32, tag="ot")
            for n in range(2):
                po = ps2.tile([128, 512], F32, tag="ps")
                for kc in range(22):
                    nc.tensor.matmul(po[:], actT[:, kc, :],
                                     w2[:, kc, n * 512:(n + 1) * 512],
                                     start=(kc == 0), stop=(kc == 21))
                nc.vector.tensor_add(ot[:, n * 512:(n + 1) * 512], po[:],
                                     ht[:, n * 512:(n + 1) * 512])
            nc.sync.dma_start(out=out_d[tt * 128:(tt + 1) * 128, :], in_=ot[:])

    nc.compile()
    return nc





def _get(name, builder, Ttok):
    key = (name, Ttok)
    if key not in _cache:
        _cache[key] = builder(Ttok)
    return _cache[key]


# ----------------------------------------------------------------------------
# Host driver
# ----------------------------------------------------------------------------
_cache = {}
LAST = {}


def host_prep_k1(ins):
    anw = f32(ins["attn_norm_w"])
    in1 = []
    for c in range(8):
        b, hg = c // 2, c % 2
        hs = slice(hg * HL, hg * HL + HL)
        qk = slice(hg * 384, hg * 384 + 384)
        vg = slice(hg * 768, hg * 768 + 768)
        wq = f32(ins["wq"][:, qk]) * anw[:, None]
        wk = f32(ins["wk"][:, qk]) * anw[:, None]
        wv = f32(ins["wv"][:, vg]) * anw[:, None]
        wg = f32(ins["wg"][:, vg]) * anw[:, None]
        wb = f32(ins["wb"][:, hs]) * anw[:, None]
        wa = f32(ins["wa"][:, hs]) * anw[:, None]
        wba = np.zeros((DIM, 38), np.float32)
        wba[:, 0:6] = wb
        wba[:, 32:38] = wa
        wba_hi = bf(wba)
        walo = wba - f32(wba_hi)
        walo[:, 0:6] = 0.0
        wcat = np.concatenate([bf(wq), bf(wk), bf(wv), bf(wg), wba_hi], axis=1)
        convw = np.concatenate([f32(ins["conv_q"][qk]), f32(ins["conv_k"][qk]),
                                f32(ins["conv_v"][vg])], axis=0)
        dtb = np.zeros((38, 1), np.float32)
        dtb[32:38, 0] = f32(ins["dt_bias"][hs])
        negA = np.zeros((38, 1), np.float32)
        negA[32:38, 0] = -np.exp(f32(ins["A_log"][hs]))
        in1.append({
            "x": f32(ins["x"][b]),
            "wcat": wcat,
            "wbahi": wba_hi,
            "walo": bf(walo),
            "convw": convw,
            "dtb": dtb,
            "negA": negA,
            "onw": f32(ins["o_norm_w"]).reshape(128, 1),
            "wo": bf(ins["wo"][hg * 768:(hg + 1) * 768, :]),
        })
    return in1


def host_prep_k2(ins, hflat, nshard=8):
    pk2 = (id(ins["w1"]), id(ins["w3"]), id(ins["w2"]))
    if _cache.get("pk2") == pk2:
        w13b, w2b = _cache["w13b"], _cache["w2b"]
    else:
        fnw = f32(ins["ffn_norm_w"])
        w1 = f32(ins["w1"]) * fnw[:, None]
        w3 = f32(ins["w3"]) * fnw[:, None]
        w13 = np.empty((DIM, 2 * FFN), np.float32)
        for j in range(FFN // 256):
            w13[:, j * 512:j * 512 + 256] = w1[:, j * 256:(j + 1) * 256]
            w13[:, j * 512 + 256:(j + 1) * 512] = w3[:, j * 256:(j + 1) * 256]
        w13b = bf(w13)
        w2b = bf(ins["w2"])
        _cache["pk2"], _cache["w13b"], _cache["w2b"] = pk2, w13b, w2b
    TK2 = hflat.shape[0] // nshard
    return [{"h": f32(hflat[c * TK2:(c + 1) * TK2]), "w13": w13b, "w2": w2b}
            for c in range(nshard)], TK2


def kernel(**inputs):
    ins = {k: np.asarray(v) for k, v in inputs.items()}
    pk = tuple(id(inputs[n]) for n in ("wq", "wk", "wv", "wg", "wb", "wa"))
    if _cache.get("pk") == pk:
        in1 = _cache["in1"]
        for c in range(8):
            in1[c]["x"] = f32(ins["x"][c // 2])
    else:
        in1 = host_prep_k1(ins)
        _cache["pk"] = pk
        _cache["in1"] = in1
    import time as _t
    nc1 = _get("k1", build_k1, T)
    t0 = _t.time()
    r1 = run_bass_kernel_spmd(nc1, in1, core_ids=list(range(8)))
    LAST["t_k1"] = _t.time() - t0
    LAST["r1"] = r1
    po = [r1.results[c]["po"] for c in range(8)]

    x = f32(ins["x"])
    h = np.stack([x[b] + po[2 * b] + po[2 * b + 1] for b in range(B)])
    in2, TK2 = host_prep_k2(ins, h.reshape(B * T, DIM))
    nc2 = _get("k2", build_k2, TK2)
    t0 = _t.time()
    r2 = run_bass_kernel_spmd(nc2, in2, core_ids=list(range(8)))
    LAST["t_k2"] = _t.time() - t0
    LAST["r2"] = r2
    out = np.concatenate([r2.results[c]["out"] for c in range(8)], axis=0)
    return out.reshape(B, T, DIM).astype(ins["x"].dtype)



# revision 2
# speedup vs baseline: 11.5305x; 11.5305x over previous
"""DeltaNet block kernel for 8 Trainium2 NeuronCores — fused single-NEFF version.

Sharding: core c -> (batch b = c//2, head-group hg = c%2, 6 heads each).
One kernel launch does everything on-device:
  AllGather(pair)      x halves (fp16) -> full x[b] per core
  Phase 1 (deltanet):  rmsnorm -> q/k/v/g/beta/a projections -> short conv ->
                       l2norm -> chunked gated delta rule (L=128 Neumann solve)
                       -> gated head RMSNorm -> partial o-projection -> poz
  ReduceScatter(pair)  poz -> per-core token half of (po[b,0]+po[b,1])
  Phase 2 (FFN):       h = x_half + po_half; out = h + (silu(hn@w1)*(hn@w3))@w2

Only x (fp16, sharded halves, 32MB total) goes host->device per call and the
output (fp16, 32MB) comes back; all weights + dummy output buffers live on
device across calls via a custom jit dispatch (see Runner).
"""
import os
from contextlib import ExitStack

import numpy as np

os.environ["BASS_NEVER_TRACE"] = "1"  # no NTFF hook under this axon client
import ml_dtypes

import concourse.bass as bass
import concourse.mybir as mybir
import concourse.tile as tile
from concourse import bacc, bass2jax
from concourse.masks import make_identity, make_upper_triangular

import jax
from jax.sharding import Mesh, PartitionSpec, NamedSharding
from jax.experimental.shard_map import shard_map

F32 = mybir.dt.float32
BF16 = mybir.dt.bfloat16
F16 = mybir.dt.float16
AF = mybir.ActivationFunctionType
ALU = mybir.AluOpType

B, T, DIM = 4, 4096, 1024
H, DK, DV = 12, 64, 128
HL = 6              # local heads per core
L = 128             # delta chunk length
SEG = 256           # tokens per segment
FFN = 2816
EPS = 1e-5
NCAT = 2342         # q(384) k(384) v(768) g(768) beta(6)@2304 a(6)@2336
TLOC = T // 2       # tokens per core for x-shard / FFN / output
GROUPS = [[0, 1], [2, 3], [4, 5], [6, 7]]  # (batch) pairs

bf = lambda a: np.ascontiguousarray(a).astype(ml_dtypes.bfloat16)
f32 = lambda a: np.ascontiguousarray(a, dtype=np.float32)


# ----------------------------------------------------------------------------
# Fused kernel builder
# ----------------------------------------------------------------------------
def build():
    nseg = T // SEG
    ncps = SEG // L  # chunks per segment
    nc = bacc.Bacc("TRN2", target_bir_lowering=False, debug=False, num_devices=8)

    xin_d = nc.dram_tensor("xin", [TLOC, DIM], F16, kind="ExternalInput")
    wcat_d = nc.dram_tensor("wcat", [DIM, NCAT], BF16, kind="ExternalInput")
    wbahi_d = nc.dram_tensor("wbahi", [DIM, 38], BF16, kind="ExternalInput")
    walo_d = nc.dram_tensor("walo", [DIM, 38], BF16, kind="ExternalInput")
    convw_d = nc.dram_tensor("convw", [1536, 4], F32, kind="ExternalInput")
    dtb_d = nc.dram_tensor("dtb", [38, 1], F32, kind="ExternalInput")
    negA_d = nc.dram_tensor("negA", [38, 1], F32, kind="ExternalInput")
    onw_d = nc.dram_tensor("onw", [128, 1], F32, kind="ExternalInput")
    wo_d = nc.dram_tensor("wo", [768, DIM], BF16, kind="ExternalInput")
    w13_d = nc.dram_tensor("w13", [DIM, 2 * FFN], BF16, kind="ExternalInput")
    w2_d = nc.dram_tensor("w2", [FFN, DIM], BF16, kind="ExternalInput")
    out_d = nc.dram_tensor("out", [TLOC, DIM], F16, kind="ExternalOutput")

    with tile.TileContext(nc) as tc, ExitStack() as top:
        dram = top.enter_context(tc.tile_pool(name="dram", bufs=1, space="DRAM"))
        cons = top.enter_context(tc.tile_pool(name="cons", bufs=1))

        # ---- shared constants ----
        id128f = cons.tile([128, 128], F32)
        make_identity(nc, id128f[:])
        id128b = cons.tile([128, 128], BF16)
        make_identity(nc, id128b[:])
        epsc = cons.tile([128, 1], F32)
        nc.vector.memset(epsc[:], EPS)

        # ---- DRAM intermediates ----
        xb = dram.tile([TLOC, DIM], F16)     # collective input bounce
        xg = dram.tile([T, DIM], F16)        # full x[b] after AllGather
        poz = dram.tile([T, DIM], F32)       # partial o-projection (my heads)
        prd = dram.tile([TLOC, DIM], F32)    # reduced po for my token half

        nc.gpsimd.dma_start(xb[:], xin_d[:])
        nc.gpsimd.collective_compute(
            "AllGather", ALU.bypass, replica_groups=GROUPS,
            ins=[xb.opt()], outs=[xg.opt()])

        # ==================== Phase 1: deltanet ====================
        with ExitStack() as ctx:
            wgt = ctx.enter_context(tc.tile_pool(name="wgt", bufs=1))
            xp = ctx.enter_context(tc.tile_pool(name="xp", bufs=2))
            segp = ctx.enter_context(tc.tile_pool(name="segp", bufs=2))
            segq = ctx.enter_context(tc.tile_pool(name="segq", bufs=1))
            ch = ctx.enter_context(tc.tile_pool(name="ch", bufs=3))
            sp = ctx.enter_context(tc.tile_pool(name="sp", bufs=1))
            psA = ctx.enter_context(tc.tile_pool(name="psA", bufs=1, space="PSUM"))
            ps19p = ctx.enter_context(tc.tile_pool(name="ps19", bufs=1, space="PSUM"))
            psB = ctx.enter_context(tc.tile_pool(name="psB", bufs=1, space="PSUM"))
            _pctr = [0]

            def pstile(dtype=F32):
                t = psB.tile([128, 256], dtype, tag=f"ps{_pctr[0] % 6}",
                             name=f"psr{_pctr[0]}")
                _pctr[0] += 1
                return t
            drp = ctx.enter_context(tc.tile_pool(name="drp", bufs=2, space="DRAM"))

            # ---- phase-1 constants ----
            mku_s = wgt.tile([128, 128], F32)   # strict upper ones
            make_upper_triangular(nc, mku_s[:], val=1.0, diag=False)
            mku_i = wgt.tile([128, 128], F32)   # inclusive upper ones
            make_upper_triangular(nc, mku_i[:], val=1.0, diag=True)
            blk2 = wgt.tile([128, 2], F32)
            nc.vector.memset(blk2[:], 0.0)
            nc.vector.memset(blk2[0:64, 0:1], 1.0)
            nc.vector.memset(blk2[64:128, 1:2], 1.0)
            zero12 = wgt.tile([38, 128], F32)
            nc.vector.memset(zero12[:], 0.0)
            epsq = wgt.tile([128, 1], F32)
            nc.vector.memset(epsq[:], float(DK) * 1e-6)
            epsk = wgt.tile([128, 1], F32)
            nc.vector.memset(epsk[:], 1e-6)

            # ---- weights to SBUF ----
            wcat = wgt.tile([128, 8, NCAT], BF16)
            nc.sync.dma_start(out=wcat[:], in_=wcat_d[:].rearrange("(a p) c -> p a c", p=128))
            wbahi = wgt.tile([128, 8, 38], BF16)
            nc.sync.dma_start(out=wbahi[:], in_=wbahi_d[:].rearrange("(a p) c -> p a c", p=128))
            walo = wgt.tile([128, 8, 38], BF16)
            nc.sync.dma_start(out=walo[:], in_=walo_d[:].rearrange("(a p) c -> p a c", p=128))
            convw = wgt.tile([128, 12, 4], F32)
            nc.sync.dma_start(out=convw[:], in_=convw_d[:].rearrange("(a p) c -> p a c", p=128))
            dtb = wgt.tile([38, 1], F32)
            nc.sync.dma_start(out=dtb[:], in_=dtb_d[:])
            negA = wgt.tile([38, 1], F32)
            nc.sync.dma_start(out=negA[:], in_=negA_d[:])
            onw = wgt.tile([128, 1], F32)
            nc.sync.dma_start(out=onw[:], in_=onw_d[:])
            wo = wgt.tile([128, 6, DIM], BF16)
            nc.sync.dma_start(out=wo[:], in_=wo_d[:].rearrange("(a p) c -> p a c", p=128))

            # persistent delta states (ping-pong per head)
            S = [[sp.tile([64, DV], BF16, tag=f"S{h}_{pp}", name=f"S{h}_{pp}")
                  for pp in range(2)] for h in range(HL)]
            for h in range(HL):
                nc.vector.memset(S[h][0][:], 0.0)

            # conv halo carry
            halo = sp.tile([128, 12, 3], BF16, tag="halo")
            nc.vector.memset(halo[:], 0.0)

            for s in range(nseg):
                # ============ x load + rmsnorm + transpose ============
                xnTh = segp.tile([128, 8, SEG], BF16, tag="xnTh")
                xnTl = segq.tile([128, 8, SEG], BF16, tag="xnTl")
                for t4 in range(SEG // 128):
                    tt = s * (SEG // 128) + t4
                    xt = xp.tile([128, DIM], F16, tag="xt")
                    nc.sync.dma_start(out=xt[:], in_=xg[tt * 128:(tt + 1) * 128, :])
                    xsq = xp.tile([128, DIM], F32, tag="xsq")
                    ssq = xp.tile([128, 1], F32, tag="ssq")
                    nc.scalar.activation(out=xsq[:], in_=xt[:], func=AF.Square,
                                         accum_out=ssq[:])
                    rst = xp.tile([128, 1], F32, tag="rst")
                    nc.scalar.activation(out=rst[:], in_=ssq[:], func=AF.Ln,
                                         scale=1.0 / DIM, bias=epsc[:])
                    nc.scalar.activation(out=rst[:], in_=rst[:], func=AF.Exp,
                                         scale=-0.5)
                    xn = xp.tile([128, DIM], F32, tag="xn")
                    nc.scalar.activation(out=xn[:], in_=xt[:], func=AF.Copy, scale=rst[:])
                    for kc in range(8):
                        pt = pstile(F32)
                        nc.tensor.transpose(pt[:, 0:128], xn[:, kc * 128:(kc + 1) * 128],
                                            id128f[:])
                        cs = slice(t4 * 128, t4 * 128 + 128)
                        nc.scalar.activation(out=xnTh[:, kc, cs], in_=pt[:, 0:128],
                                             func=AF.Copy)
                        nc.vector.tensor_sub(xnTl[:, kc, cs], pt[:, 0:128],
                                             xnTh[:, kc, cs])

                # ============ projections ============
                qkvb = segq.tile([128, 12, SEG + 3], BF16, tag="qkvb")
                nc.scalar.activation(out=qkvb[:, :, 0:3], in_=halo[:], func=AF.Copy)
                gateT = segq.tile([128, 6, SEG], BF16, tag="gateT")
                for jcol in range(18):
                    c0 = jcol * 128
                    pj = psA.tile([128, SEG], F32, tag="psA")
                    for kc in range(8):
                        nc.tensor.matmul(pj[:], wcat[:, kc, c0:c0 + 128],
                                         xnTh[:, kc, :], start=(kc == 0), stop=(kc == 7))
                    if jcol < 12:
                        nc.scalar.activation(out=qkvb[:, jcol, 3:SEG + 3], in_=pj[:],
                                             func=AF.Copy)
                    else:
                        nc.scalar.activation(out=gateT[:, jcol - 12, :], in_=pj[:],
                                             func=AF.Silu)
                # beta/a columns with low-precision corrections
                p19 = ps19p.tile([38, SEG], F32, tag="p19")
                for kc in range(8):
                    nc.tensor.matmul(p19[:], wcat[:, kc, 2304:2342], xnTh[:, kc, :],
                                     start=(kc == 0), stop=False)
                for kc in range(8):
                    nc.tensor.matmul(p19[:], wbahi[:, kc, :], xnTl[:, kc, :],
                                     start=False, stop=False)
                for kc in range(8):
                    nc.tensor.matmul(p19[:], walo[:, kc, :], xnTh[:, kc, :],
                                     start=False, stop=(kc == 7))
                ba = segq.tile([38, SEG], F32, tag="ba")
                nc.scalar.activation(out=ba[:], in_=p19[:], func=AF.Copy)

                # ============ conv + silu ============
                csil = segp.tile([128, 12, SEG], BF16, tag="csil")
                cacc = segq.tile([128, 12, SEG], BF16, tag="cacc")
                ctmp = segq.tile([128, 12, SEG], BF16, tag="ctmp")
                nc.vector.tensor_mul(cacc[:], qkvb[:, :, 3:SEG + 3],
                                     convw[:, :, 3:4].to_broadcast((128, 12, SEG)))
                for i in (2, 1, 0):
                    nc.vector.tensor_mul(ctmp[:], qkvb[:, :, i:i + SEG],
                                         convw[:, :, i:i + 1].to_broadcast((128, 12, SEG)))
                    nc.vector.tensor_add(cacc[:], cacc[:], ctmp[:])
                nc.scalar.activation(out=halo[:], in_=qkvb[:, :, SEG:SEG + 3], func=AF.Copy)
                nc.scalar.activation(out=csil[:], in_=cacc[:], func=AF.Silu)

                # ============ l2norm scales for q/k ============
                sqt = segq.tile([128, SEG], F32, tag="sqt")
                rp = []
                for t in range(6):
                    nc.scalar.activation(out=sqt[:], in_=csil[:, t, :], func=AF.Square)
                    pq = pstile(F32)
                    nc.tensor.matmul(pq[0:2, 0:SEG], blk2[:], sqt[:],
                                     start=True, stop=True)
                    rpt = segp.tile([2, SEG], F32, tag=f"rp{t}", name=f"rp{t}")
                    if t < 3:
                        nc.scalar.activation(out=rpt[:], in_=pq[0:2, 0:SEG], func=AF.Ln,
                                             scale=float(DK), bias=epsq[0:2, :])
                    else:
                        nc.scalar.activation(out=rpt[:], in_=pq[0:2, 0:SEG], func=AF.Ln,
                                             scale=1.0, bias=epsk[0:2, :])
                    nc.scalar.activation(out=rpt[:], in_=rpt[:], func=AF.Exp,
                                         scale=-0.5)
                    rp.append(rpt)

                # plain-scaled q/k (channel-major)
                Qts = segp.tile([128, 3, SEG], BF16, tag="Qts")
                Kts = segp.tile([128, 3, SEG], BF16, tag="Kts")
                bcq = segq.tile([128, SEG], F32, tag="bcq")
                bck = segq.tile([128, SEG], F32, tag="bck")
                for t in range(3):
                    rqd = drp.tile([2, SEG], F32, tag="rqd")
                    nc.sync.dma_start(out=rqd[:], in_=rp[t][:])
                    rkd = drp.tile([2, SEG], F32, tag="rkd")
                    nc.sync.dma_start(out=rkd[:], in_=rp[3 + t][:])
                    for i in range(2):
                        hh = slice(64 * i, 64 * i + 64)
                        nc.sync.dma_start(out=bcq[hh, :], in_=rqd[i:i + 1, :].to_broadcast((64, SEG)))
                        nc.sync.dma_start(out=bck[hh, :], in_=rkd[i:i + 1, :].to_broadcast((64, SEG)))
                    nc.vector.tensor_mul(Qts[:, t, :], csil[:, t, :], bcq[:])
                    nc.vector.tensor_mul(Kts[:, t, :], csil[:, 3 + t, :], bck[:])

                # ============ delta chunks ============
                gato = segp.tile([128, 6, SEG], BF16, tag="gato")
                for cc in range(ncps):
                    csl = slice(cc * L, (cc + 1) * L)
                    cglob = s * ncps + cc

                    # ---- beta / g / gc pipeline for this chunk ----
                    spg = ch.tile([38, 128], F32, tag="spg")
                    gcsg = ch.tile([38, 128], F32, tag="gcsg")
                    nc.scalar.activation(out=gcsg[0:6, :], in_=ba[0:6, csl],
                                         func=AF.Exp, scale=-1.0)
                    nc.vector.tensor_scalar(out=gcsg[0:6, :], in0=gcsg[0:6, :],
                                            scalar1=1.0, scalar2=None, op0=ALU.add)
                    nc.vector.reciprocal(out=gcsg[0:6, :], in_=gcsg[0:6, :])
                    nc.scalar.activation(out=spg[32:38, :], in_=ba[32:38, csl],
                                         func=AF.Exp, bias=dtb[32:38, :])
                    nc.scalar.activation(out=spg[32:38, :], in_=spg[32:38, :],
                                         func=AF.Ln, bias=1.0)
                    grow = ch.tile([38, 128], F32, tag="grow")
                    nc.vector.tensor_scalar(out=grow[32:38, :], in0=spg[32:38, :],
                                            scalar1=negA[32:38, :], scalar2=None,
                                            op0=ALU.mult)
                    nc.vector.tensor_tensor_scan(out=gcsg[32:38, :], data0=grow[32:38, :],
                                                 data1=zero12[32:38, :], initial=0.0,
                                                 op0=ALU.add, op1=ALU.add)
                    ptb = pstile(F32)
                    nc.tensor.transpose(ptb[:, 0:38], gcsg[:], id128f[0:38, 0:38])
                    bgt = ch.tile([128, 38], F32, tag="bgt")
                    nc.scalar.activation(out=bgt[:], in_=ptb[:, 0:38], func=AF.Copy)
                    # gc rows to DRAM once; replicate rows and last-token column back
                    gcd = drp.tile([6, 128], F32, tag="gcd")
                    nc.sync.dma_start(out=gcd[:], in_=gcsg[32:38, :])
                    gcrep6 = ch.tile([128, 6, 128], F32, tag="gcrep6")
                    nc.sync.dma_start(
                        out=gcrep6[:],
                        in_=bass.AP(tensor=gcd.tensor, offset=gcd.offset,
                                    ap=[[0, 128], [128, 6], [1, 128]]))
                    gamc = ch.tile([128, 6], F32, tag="gamc")
                    nc.scalar.activation(out=gamc[:], in_=bgt[:, 32:38], func=AF.Exp)
                    gclr = ch.tile([128, 6], F32, tag="gclr")
                    nc.sync.dma_start(
                        out=gclr[:],
                        in_=bass.AP(tensor=gcd.tensor, offset=gcd.offset + 127,
                                    ap=[[0, 128], [128, 6]]))
                    dtmp = ch.tile([128, 6], F32, tag="dtmp")
                    nc.vector.tensor_sub(dtmp[:], gclr[:], bgt[:, 32:38])
                    dcola = ch.tile([128, 6], F32, tag="dcola")
                    nc.scalar.activation(out=dcola[:], in_=dtmp[:], func=AF.Exp)
                    gamls = ch.tile([128, 6], F32, tag="gamls")
                    nc.scalar.activation(out=gamls[:], in_=gclr[:], func=AF.Exp)

                    # q/k token-major pairs
                    ktokp = ch.tile([128, 3, 128], BF16, tag="ktokp")
                    qtokp = ch.tile([128, 3, 128], BF16, tag="qtokp")
                    for t in range(3):
                        pkt = pstile(BF16)
                        nc.tensor.transpose(pkt[:, 0:128], Kts[:, t, csl], id128b[:])
                        nc.scalar.activation(out=ktokp[:, t, :], in_=pkt[:, 0:128],
                                             func=AF.Copy)
                        pqt = pstile(BF16)
                        nc.tensor.transpose(pqt[:, 0:128], Qts[:, t, csl], id128b[:])
                        nc.scalar.activation(out=qtokp[:, t, :], in_=pqt[:, 0:128],
                                             func=AF.Copy)
                    # Gamma-scaled q, back to channel-major at partition base 0
                    qgch = []
                    for h2 in range(HL):
                        t2, half2 = h2 // 2, h2 % 2
                        qtg = ch.tile([128, 64], BF16, tag="qtg", name="qtg")
                        nc.vector.tensor_scalar(out=qtg[:],
                                                in0=qtokp[:, t2, 64 * half2:64 * half2 + 64],
                                                scalar1=gamc[:, h2:h2 + 1], scalar2=None,
                                                op0=ALU.mult)
                        pqg = pstile(BF16)
                        nc.tensor.transpose(pqg[0:64, 0:128], qtg[:], id128b[:])
                        qg = ch.tile([64, 128], BF16, tag=f"qg{h2}", name=f"qg{h2}")
                        nc.scalar.activation(out=qg[:], in_=pqg[0:64, 0:128], func=AF.Copy)
                        qgch.append(qg)

                    for h in range(HL):
                        t, half = h // 2, h % 2
                        hh = slice(64 * half, 64 * half + 64)
                        Ksl = Kts[hh, t, csl]
                        Qsl = Qts[hh, t, csl]
                        Qgsl = qgch[h][:]
                        Ktok = ktokp[:, t, 64 * half:64 * half + 64]
                        Sprev = S[h][cglob % 2]
                        Snext = S[h][(cglob + 1) % 2]

                        # masked KK^T and KQ^T
                        pkk = pstile(F32)
                        nc.tensor.matmul(pkk[:, 0:128], Ksl, Ksl, start=True, stop=True)
                        Msb = ch.tile([128, 128], F32, tag="Msb")
                        nc.vector.tensor_mul(Msb[:], mku_s[:], pkk[:, 0:128])
                        pkq = pstile(F32)
                        nc.tensor.matmul(pkq[:, 0:128], Ksl, Qsl, start=True, stop=True)
                        KQm = ch.tile([128, 128], F32, tag="KQm")
                        nc.vector.tensor_mul(KQm[:], mku_i[:], pkq[:, 0:128])

                        # decay matrix Db[i,t] = exp(min(gc_t - gc_i, 0))
                        Db = ch.tile([128, 128], F32, tag="Db")
                        nc.vector.tensor_scalar(out=Db[:], in0=gcrep6[:, h, :],
                                                scalar1=bgt[:, 32 + h:33 + h],
                                                scalar2=0.0, op0=ALU.subtract,
                                                op1=ALU.min)
                        nc.scalar.activation(out=Db[:], in_=Db[:], func=AF.Exp)

                        # Abar = beta_i * Db * M ; Gbar = Db * KQ
                        Ab = ch.tile([128, 128], BF16, tag="Ab")
                        nc.vector.scalar_tensor_tensor(out=Ab[:], in0=Db[:],
                                                       scalar=bgt[:, h:h + 1], in1=Msb[:],
                                                       op0=ALU.mult, op1=ALU.mult)
                        Gb = ch.tile([128, 128], BF16, tag="Gb")
                        nc.vector.tensor_mul(Gb[:], Db[:], KQm[:])

                        # 16-term Neumann inverse factors
                        pw = pstile(BF16)
                        At = ch.tile([128, 128], BF16, tag="At")
                        nc.tensor.transpose(pw[:, 0:128], Ab[:], id128b[:])
                        nc.scalar.activation(out=At[:], in_=pw[:, 0:128], func=AF.Copy)
                        pw2 = pstile(F32)
                        nc.tensor.matmul(pw2[:, 0:128], At[:], Ab[:], start=True, stop=True)
                        A2p = ch.tile([128, 128], BF16, tag="A2p")
                        A2i = ch.tile([128, 128], BF16, tag="A2i")
                        nc.scalar.activation(out=A2p[:], in_=pw2[:, 0:128], func=AF.Copy)
                        nc.vector.tensor_add(A2i[:], id128b[:], pw2[:, 0:128])
                        pw3 = pstile(F32)
                        nc.tensor.matmul(pw3[:, 0:128], Ab[:], At[:], start=True, stop=True)
                        T2p = ch.tile([128, 128], BF16, tag="T2p")
                        nc.scalar.activation(out=T2p[:], in_=pw3[:, 0:128], func=AF.Copy)
                        pw4 = pstile(F32)
                        nc.tensor.matmul(pw4[:, 0:128], T2p[:], A2p[:], start=True, stop=True)
                        A4p = ch.tile([128, 128], BF16, tag="A4p")
                        A4i = ch.tile([128, 128], BF16, tag="A4i")
                        nc.scalar.activation(out=A4p[:], in_=pw4[:, 0:128], func=AF.Copy)
                        nc.vector.tensor_add(A4i[:], id128b[:], pw4[:, 0:128])
                        pw5 = pstile(F32)
                        nc.tensor.matmul(pw5[:, 0:128], A2p[:], T2p[:], start=True, stop=True)
                        T4p = ch.tile([128, 128], BF16, tag="T4p")
                        nc.scalar.activation(out=T4p[:], in_=pw5[:, 0:128], func=AF.Copy)
                        pw6 = pstile(F32)
                        nc.tensor.matmul(pw6[:, 0:128], T4p[:], A4p[:], start=True, stop=True)
                        A8i = ch.tile([128, 128], BF16, tag="A8i")
                        nc.vector.tensor_add(A8i[:], id128b[:], pw6[:, 0:128])
                        F0 = ch.tile([128, 128], BF16, tag="F0")
                        nc.vector.tensor_sub(F0[:], id128b[:], Ab[:])

                        # X0 = [Vtok | Ktok*Gamma]
                        X0 = ch.tile([128, 192], BF16, tag="X0")
                        pvt = pstile(BF16)
                        nc.tensor.transpose(pvt[:, 0:128], csil[:, 6 + h, csl], id128b[:])
                        nc.scalar.activation(out=X0[:, 0:128], in_=pvt[:, 0:128],
                                             func=AF.Copy)
                        nc.vector.tensor_scalar(out=X0[:, 128:192], in0=Ktok,
                                                scalar1=gamc[:, h:h + 1], scalar2=None,
                                                op0=ALU.mult)

                        # apply chain: X4 = (I-A)(I+A2)(I+A4)(I+A8) X0
                        px1 = pstile(F32)
                        nc.tensor.matmul(px1[:, 0:192], A8i[:], X0[:], start=True, stop=True)
                        X1 = ch.tile([128, 192], BF16, tag="X1")
                        nc.scalar.activation(out=X1[:], in_=px1[:, 0:192], func=AF.Copy)
                        px2 = pstile(F32)
                        nc.tensor.matmul(px2[:, 0:192], A4i[:], X1[:], start=True, stop=True)
                        X2 = ch.tile([128, 192], BF16, tag="X2")
                        nc.vector.tensor_copy(X2[:], px2[:, 0:192])
                        px3 = pstile(F32)
                        nc.tensor.matmul(px3[:, 0:192], A2i[:], X2[:], start=True, stop=True)
                        X3 = ch.tile([128, 192], BF16, tag="X3")
                        nc.scalar.activation(out=X3[:], in_=px3[:, 0:192], func=AF.Copy)
                        px4 = pstile(F32)
                        nc.tensor.matmul(px4[:, 0:192], F0[:], X3[:], start=True, stop=True)
                        YJb = ch.tile([128, 192], BF16, tag="YJb")
                        nc.scalar.activation(out=YJb[:], in_=px4[:, 0:192], func=AF.Copy,
                                             scale=bgt[:, h:h + 1])

                        # U = Yb - Jb S0
                        pjt = pstile(BF16)
                        nc.tensor.transpose(pjt[0:64, 0:128], YJb[:, 128:192], id128b[:])
                        nJT = ch.tile([64, 128], BF16, tag="nJT")
                        nc.scalar.activation(out=nJT[:], in_=pjt[0:64, 0:128],
                                             func=AF.Copy, scale=-1.0)
                        pU = pstile(F32)
                        nc.tensor.matmul(pU[:, 0:128], nJT[:], Sprev[:], start=True,
                                         stop=True)
                        Usb = ch.tile([128, 128], BF16, tag="Usb")
                        nc.vector.tensor_add(Usb[:], pU[:, 0:128], YJb[:, 0:128])

                        # O = Qg S0 + G U (token-major), normalize, gate
                        pO = pstile(F32)
                        nc.tensor.matmul(pO[:, 0:128], Qgsl, Sprev[:], start=True,
                                         stop=False)
                        nc.tensor.matmul(pO[:, 0:128], Gb[:], Usb[:], start=False,
                                         stop=True)
                        osc = ch.tile([128, 128], F32, tag="osc")
                        ossq = ch.tile([128, 1], F32, tag="ossq")
                        nc.scalar.activation(out=osc[:], in_=pO[:, 0:128], func=AF.Square,
                                             accum_out=ossq[:])
                        orst = ch.tile([128, 1], F32, tag="orst")
                        nc.scalar.activation(out=orst[:], in_=ossq[:], func=AF.Ln,
                                             scale=1.0 / DV, bias=epsc[:])
                        nc.scalar.activation(out=orst[:], in_=orst[:], func=AF.Exp,
                                             scale=-0.5)
                        On = ch.tile([128, 128], BF16, tag="On")
                        nc.scalar.activation(out=On[:], in_=pO[:, 0:128], func=AF.Copy,
                                             scale=orst[:])
                        pot = pstile(BF16)
                        nc.tensor.transpose(pot[:, 0:128], On[:], id128b[:])
                        nc.vector.scalar_tensor_tensor(out=gato[:, h, csl],
                                                       in0=pot[:, 0:128], scalar=onw[:],
                                                       in1=gateT[:, h, csl],
                                                       op0=ALU.mult, op1=ALU.mult)

                        # S update: Snext = GamL*Sprev + Kbar^T U
                        Kb = ch.tile([128, 64], BF16, tag="Kb")
                        nc.vector.tensor_scalar(out=Kb[:], in0=Ktok,
                                                scalar1=dcola[:, h:h + 1], scalar2=None,
                                                op0=ALU.mult)
                        pS = pstile(F32)
                        nc.tensor.matmul(pS[0:64, 0:128], Kb[:], Usb[:], start=True,
                                         stop=True)
                        nc.vector.scalar_tensor_tensor(out=Snext[:], in0=Sprev[:],
                                                       scalar=gamls[0:64, h:h + 1],
                                                       in1=pS[0:64, 0:128],
                                                       op0=ALU.mult, op1=ALU.add)

                # ============ o-projection ============
                for t4 in range(SEG // 128):
                    tsl = slice(t4 * 128, t4 * 128 + 128)
                    tt = s * (SEG // 128) + t4
                    post = xp.tile([128, DIM], F32, tag="post")
                    for n in range(2):
                        pp = psA.tile([128, 512], F32, tag="psA")
                        for j in range(6):
                            nc.tensor.matmul(pp[:], gato[:, j, tsl],
                                             wo[:, j, n * 512:(n + 1) * 512],
                                             start=(j == 0), stop=(j == 5))
                        nc.scalar.activation(out=post[:, n * 512:(n + 1) * 512],
                                             in_=pp[:], func=AF.Copy)
                    nc.sync.dma_start(out=poz[tt * 128:(tt + 1) * 128, :], in_=post[:])

        # ==================== pair-reduce po ====================
        nc.gpsimd.collective_compute(
            "ReduceScatter", ALU.add, replica_groups=GROUPS,
            ins=[poz.opt()], outs=[prd.opt()])

        # ==================== Phase 2: FFN ====================
        with ExitStack() as ctx:
            wgt2 = ctx.enter_context(tc.tile_pool(name="wgt2", bufs=1))
            tp = ctx.enter_context(tc.tile_pool(name="tp", bufs=2))
            ps1 = ctx.enter_context(tc.tile_pool(name="ps1", bufs=4, space="PSUM"))
            ps2 = ctx.enter_context(tc.tile_pool(name="ps2", bufs=2, space="PSUM"))
            NB = FFN // 256  # 11 paired column blocks

            w13 = wgt2.tile([128, 8, 2 * FFN], BF16)
            nc.sync.dma_start(out=w13[:], in_=w13_d[:].rearrange("(a p) c -> p a c", p=128))
            w2 = wgt2.tile([128, 22, DIM], BF16)
            nc.sync.dma_start(out=w2[:], in_=w2_d[:].rearrange("(a p) c -> p a c", p=128))

            for tt in range(TLOC // 128):
                xt16 = tp.tile([128, DIM], F16, tag="xt16")
                nc.sync.dma_start(out=xt16[:], in_=xin_d[tt * 128:(tt + 1) * 128, :])
                prt = tp.tile([128, DIM], F32, tag="prt")
                nc.sync.dma_start(out=prt[:], in_=prd[tt * 128:(tt + 1) * 128, :])
                ht = tp.tile([128, DIM], F32, tag="ht")
                nc.vector.tensor_add(ht[:], prt[:], xt16[:])
                hsq = tp.tile([128, DIM], F32, tag="hsq")
                ssq = tp.tile([128, 1], F32, tag="ssq")
                nc.scalar.activation(out=hsq[:], in_=ht[:], func=AF.Square,
                                     accum_out=ssq[:])
                rst = tp.tile([128, 1], F32, tag="rst")
                nc.scalar.activation(out=rst[:], in_=ssq[:], func=AF.Ln,
                                     scale=1.0 / DIM, bias=epsc[:])
                nc.scalar.activation(out=rst[:], in_=rst[:], func=AF.Exp,
                                     scale=-0.5)
                hn = tp.tile([128, DIM], F32, tag="hn")
                nc.scalar.activation(out=hn[:], in_=ht[:], func=AF.Copy, scale=rst[:])
                hnT = tp.tile([128, 8, 128], BF16, tag="hnT")
                for kc in range(8):
                    pt = ps1.tile([128, 256], F32, tag="ps")
                    nc.tensor.transpose(pt[:, 0:128], hn[:, kc * 128:(kc + 1) * 128],
                                        id128f[:])
                    nc.scalar.activation(out=hnT[:, kc, :], in_=pt[:, 0:128], func=AF.Copy)

                act = tp.tile([128, FFN], BF16, tag="act")
                for j in range(NB):
                    p1 = ps1.tile([128, 256], F32, tag="ps")
                    p3 = ps1.tile([128, 256], F32, tag="ps")
                    c0 = j * 512
                    for kc in range(8):
                        nc.tensor.matmul(p1[:], hnT[:, kc, :], w13[:, kc, c0:c0 + 256],
                                         start=(kc == 0), stop=(kc == 7))
                    for kc in range(8):
                        nc.tensor.matmul(p3[:], hnT[:, kc, :],
                                         w13[:, kc, c0 + 256:c0 + 512],
                                         start=(kc == 0), stop=(kc == 7))
                    sl1 = tp.tile([128, 256], BF16, tag="sl1")
                    nc.scalar.activation(out=sl1[:], in_=p1[:], func=AF.Silu)
                    nc.vector.scalar_tensor_tensor(out=act[:, j * 256:(j + 1) * 256],
                                                   in0=p3[:], scalar=1.0, in1=sl1[:],
                                                   op0=ALU.mult, op1=ALU.mult)
                actT = tp.tile([128, 22, 128], BF16, tag="actT")
                for kc in range(22):
                    pt = ps1.tile([128, 256], BF16, tag="ps")
                    nc.tensor.transpose(pt[:, 0:128], act[:, kc * 128:(kc + 1) * 128],
                                        id128b[:])
                    nc.scalar.activation(out=actT[:, kc, :], in_=pt[:, 0:128],
                                         func=AF.Copy)
                ot = tp.tile([128, DIM], F16, tag="ot")
                for n in range(2):
                    po = ps2.tile([128, 512], F32, tag="ps")
                    for kc in range(22):
                        nc.tensor.matmul(po[:], actT[:, kc, :],
                                         w2[:, kc, n * 512:(n + 1) * 512],
                                         start=(kc == 0), stop=(kc == 21))
                    nc.vector.tensor_add(ot[:, n * 512:(n + 1) * 512], po[:],
                                         ht[:, n * 512:(n + 1) * 512])
                nc.sync.dma_start(out=out_d[tt * 128:(tt + 1) * 128, :], in_=ot[:])

    nc.compile()
    return nc


# ----------------------------------------------------------------------------
# Custom cached PJRT dispatch: statics stay on device across calls
# ----------------------------------------------------------------------------
class Runner:
    def __init__(self, nc, n_cores=8):
        bass2jax.install_neuronx_cc_hook()
        pname = nc.partition_id_tensor.name if nc.partition_id_tensor else None
        in_names, out_names, out_avals = [], [], []
        for alloc in nc.m.functions[0].allocations:
            if not isinstance(alloc, mybir.MemoryLocationSet):
                continue
            name = alloc.memorylocations[0].name
            if alloc.kind == "ExternalInput":
                if name != pname:
                    in_names.append(name)
            elif alloc.kind == "ExternalOutput":
                shape = tuple(alloc.tensor_shape)
                dtype = mybir.dt.np(alloc.dtype)
                out_names.append(name)
                out_avals.append(jax.core.ShapedArray(shape, dtype))
        self.in_names = in_names
        self.out_names = out_names
        self.out_avals = out_avals
        all_in = in_names + out_names + ([pname] if pname else [])
        n_op = len(in_names) + len(out_names)

        def _body(*args):
            operands = list(args)
            if pname is not None:
                operands.append(bass2jax.partition_id_tensor())
            outs = bass2jax._bass_exec_p.bind(
                *operands,
                out_avals=tuple(out_avals),
                in_names=tuple(all_in),
                out_names=tuple(out_names),
                lowering_input_output_aliases=(),
                sim_require_finite=True,
                sim_require_nnan=True,
                nc=nc,
            )
            return tuple(outs)

        self.mesh = Mesh(np.asarray(jax.devices()[:n_cores]), ("core",))
        self.sharding = NamedSharding(self.mesh, PartitionSpec("core"))
        self.f = jax.jit(
            shard_map(_body, mesh=self.mesh,
                      in_specs=(PartitionSpec("core"),) * n_op,
                      out_specs=(PartitionSpec("core"),) * len(out_names),
                      check_rep=False),
            keep_unused=True)

    def put(self, arr):
        return jax.device_put(arr, self.sharding)


# ----------------------------------------------------------------------------
# Host driver
# ----------------------------------------------------------------------------
_cache = {}
LAST = {}


def host_prep_statics(ins):
    """Per-core weight arrays, concatenated core-major for the sharded jit."""
    anw = f32(ins["attn_norm_w"])
    fnw = f32(ins["ffn_norm_w"])
    w1 = f32(ins["w1"]) * fnw[:, None]
    w3 = f32(ins["w3"]) * fnw[:, None]
    w13 = np.empty((DIM, 2 * FFN), np.float32)
    for j in range(FFN // 256):
        w13[:, j * 512:j * 512 + 256] = w1[:, j * 256:(j + 1) * 256]
        w13[:, j * 512 + 256:(j + 1) * 512] = w3[:, j * 256:(j + 1) * 256]
    w13b = bf(w13)
    w2b = bf(ins["w2"])

    per_core = []
    for c in range(8):
        hg = c % 2
        hs = slice(hg * HL, hg * HL + HL)
        qk = slice(hg * 384, hg * 384 + 384)
        vg = slice(hg * 768, hg * 768 + 768)
        wq = f32(ins["wq"][:, qk]) * anw[:, None]
        wk = f32(ins["wk"][:, qk]) * anw[:, None]
        wv = f32(ins["wv"][:, vg]) * anw[:, None]
        wg = f32(ins["wg"][:, vg]) * anw[:, None]
        wb = f32(ins["wb"][:, hs]) * anw[:, None]
        wa = f32(ins["wa"][:, hs]) * anw[:, None]
        wba = np.zeros((DIM, 38), np.float32)
        wba[:, 0:6] = wb
        wba[:, 32:38] = wa
        wba_hi = bf(wba)
        walo = wba - f32(wba_hi)
        walo[:, 0:6] = 0.0
        wcat = np.concatenate([bf(wq), bf(wk), bf(wv), bf(wg), wba_hi], axis=1)
        convw = np.concatenate([f32(ins["conv_q"][qk]), f32(ins["conv_k"][qk]),
                                f32(ins["conv_v"][vg])], axis=0)
        dtb = np.zeros((38, 1), np.float32)
        dtb[32:38, 0] = f32(ins["dt_bias"][hs])
        negA = np.zeros((38, 1), np.float32)
        negA[32:38, 0] = -np.exp(f32(ins["A_log"][hs]))
        per_core.append({
            "wcat": wcat,
            "wbahi": wba_hi,
            "walo": bf(walo),
            "convw": convw,
            "dtb": dtb,
            "negA": negA,
            "onw": f32(ins["o_norm_w"]).reshape(128, 1),
            "wo": bf(ins["wo"][hg * 768:(hg + 1) * 768, :]),
            "w13": w13b,
            "w2": w2b,
        })
    return {name: np.concatenate([per_core[c][name] for c in range(8)], axis=0)
            for name in per_core[0]}


def _setup(ins):
    """Build + compile the kernel and upload statics; cached across calls."""
    if "runner" not in _cache:
        nc = build()
        _cache["runner"] = Runner(nc)
    r = _cache["runner"]
    statics = host_prep_statics(ins)
    dev = {name: r.put(arr) for name, arr in statics.items()}
    # dummy operands for the output slots (NEFF ignores them; kernel writes
    # every output element into fresh result buffers)
    for name, aval in zip(r.out_names, r.out_avals):
        z = np.zeros((8 * aval.shape[0],) + tuple(aval.shape[1:]), aval.dtype)
        dev[name] = r.put(z)
    _cache["dev"] = dev
    return r


def kernel(**inputs):
    ins = {k: np.asarray(v) for k, v in inputs.items()}
    pk = tuple(id(inputs[n]) for n in ("wq", "wk", "wv", "wg", "wb", "wa",
                                       "wo", "w1", "w3", "w2"))
    if _cache.get("pk") != pk:
        r = _setup(ins)
        _cache["pk"] = pk
    else:
        r = _cache["runner"]
    dev = _cache["dev"]

    import time as _t
    t0 = _t.time()
    xh = ins["x"].reshape(8 * TLOC, DIM).astype(np.float16)
    x_dev = r.put(xh)
    args = [x_dev if n == "xin" else dev[n] for n in r.in_names]
    args += [dev[n] for n in r.out_names]
    out = r.f(*args)
    res = np.asarray(out[0])
    LAST["t_k1"] = _t.time() - t0
    LAST["t_k2"] = 0.0
    return res.astype(np.float32).reshape(B, T, DIM).astype(ins["x"].dtype)
